# revision 1
# baseline (speedup 1.0000x reference)
"""Trainium2 Bass kernel for nn_NovaLinkPredictor (hetero GraphSAGE link predictor).

8-core SPMD strategy:
  - Users sharded by range: 8 x 25088 rows (padded 200704). Movies: 8 x 10112 (padded 80896).
  - Edges bucketed by src-range (user side) and globally dst-sorted (movie side), on host.
  - Segment-sums computed on device with one-hot scatter matmuls (S^T @ G) accumulated in
    PSUM per 128-node tile; gathers via dma_gather (int16 idx, 3-range split for movie tables).
  - conv1 movie-side aggregation degenerates: user_x rows are identical (u0), so
    mean = u0 * (cnt_m > 0); cnt_m via a count-only pass + ReduceScatter.
  - Tables exchanged between cores with AllGather / AllToAll collectives (bf16).
  - Final edge dots: labels bucketed by user-range; gather user_o (local) + movie_o (AG'd).

The device program structure (loop bounds) is derived from max-over-core chunk counts so a
single SPMD program serves all 8 cores; per-core data (indices, one-hot keys) comes via inputs.
"""
import sys
sys.path.insert(0, "/opt/trn_rl_repo")
import numpy as np
import ml_dtypes

from concourse import bass, mybir, bacc, tile
from concourse.bass_utils import run_bass_kernel_spmd
from concourse.masks import make_identity

# ---------------- constants ----------------
H = 128
NU = 200000
NM = 80000
FD = 512
W = 8
P = 128

USR = 25088            # user rows per core (196 tiles)
UT = 196
NUP = USR * W          # 200704
MSL = 10112            # movie rows per core (79 tiles)
MT = 79
NMP = MSL * W          # 80896
GMT = NMP // P         # 632 global movie tiles

RNG_STARTS = [0, 27008, 54016]          # movie gather ranges (int16-safe)
RNG_ENDS = [27008, 54016, NMP]
NRNG = 3
GROUP = 8             # chunks per dma_gather (8*128 = 1024 rows; >1024 rows crashes)
SENT = 200.0           # one-hot sentinel (outside 0..127)

bf16 = mybir.dt.bfloat16
f32 = mybir.dt.float32
f32r = mybir.dt.float32r
i16 = mybir.dt.int16
npbf16 = ml_dtypes.bfloat16


# ---------------- host-side preprocessing ----------------

def _wrap16(idx):
    """int16 stream -> [128, n/16] wrapped layout for dma_gather idxs."""
    n = idx.shape[0]
    assert n % 16 == 0
    w = idx.reshape(n // 16, 16).T.astype(np.int16)      # [16, n/16]
    return np.ascontiguousarray(np.tile(w, (8, 1)))      # [128, n/16]


def _chunk_layout(vals, n_chunks, fill):
    """[n_chunks*128] padded stream -> [128, n_chunks] (partition-major)."""
    a = np.full(n_chunks * P, fill, dtype=vals.dtype)
    a[: len(vals)] = vals
    return np.ascontiguousarray(a.reshape(n_chunks, P).T)


def _segment_streams(gidx_list, loc_list, n_cores):
    """Given per-(core) lists of per-segment (gidx, loc) arrays keyed identically,
    pad each segment to the max-over-cores chunk count. Returns per-core
    (gidx_stream, loc_stream[128, NB]) plus per-segment chunk counts."""
    nseg = len(gidx_list[0])
    seg_chunks = []
    for s in range(nseg):
        mx = max(len(gidx_list[c][s]) for c in range(n_cores))
        seg_chunks.append((mx + P - 1) // P)
    nb = sum(seg_chunks)
    g_streams, l_streams = [], []
    for c in range(n_cores):
        g = np.zeros(nb * P, np.int16)
        l = np.full(nb * P, SENT, np.float32)
        pos = 0
        for s in range(nseg):
            n = len(gidx_list[c][s])
            g[pos: pos + n] = gidx_list[c][s]
            l[pos: pos + n] = loc_list[c][s]
            pos += seg_chunks[s] * P
        g_streams.append(g)
        l_streams.append(np.ascontiguousarray(l.reshape(nb, P).T))
    return g_streams, l_streams, seg_chunks


def preprocess(edge_src, edge_dst, lbl_user, lbl_movie):
    """Shard + sort edges/labels; build device index streams and program structure."""
    S = {}
    edge_src = np.asarray(edge_src).astype(np.int64)
    edge_dst = np.asarray(edge_dst).astype(np.int64)
    lbl_user = np.asarray(lbl_user).astype(np.int64)
    lbl_movie = np.asarray(lbl_movie).astype(np.int64)

    u_core = edge_src // USR
    u_loc = edge_src - u_core * USR

    # ---- Pass B streams: per core, segments = (range r, user tile t) in r-major order ----
    B_g, B_l = [], []          # per core: list of per-segment arrays
    for c in range(W):
        m = u_core == c
        src_l = u_loc[m]
        dst = edge_dst[m]
        rng = np.minimum(dst // 27008, 2)
        tilev = src_l // P
        order = np.lexsort((dst, tilev, rng))
        src_l, dst, rng, tilev = src_l[order], dst[order], rng[order], tilev[order]
        segs_g, segs_l = [], []
        for r in range(NRNG):
            for t in range(UT):
                mm = (rng == r) & (tilev == t)
                segs_g.append((dst[mm] - RNG_STARTS[r]).astype(np.int16))
                segs_l.append((src_l[mm] - t * P).astype(np.float32))
        B_g.append(segs_g)
        B_l.append(segs_l)
    Bg_str, Bl_str, B_seg_chunks = _segment_streams(B_g, B_l, W)
    # chunks per (r, t): B_seg_chunks[r*UT + t]
    S["B_chunks"] = np.array(B_seg_chunks).reshape(NRNG, UT)
    S["NB"] = int(S["B_chunks"].sum())

    # ---- Pass A/C streams: per core, segments = global movie tile g (dst-sorted) ----
    C_g, C_l = [], []
    for c in range(W):
        m = u_core == c
        src_l = u_loc[m]
        dst = edge_dst[m]
        order = np.argsort(dst, kind="stable")
        src_l, dst = src_l[order], dst[order]
        gt = dst // P
        segs_g, segs_l = [], []
        for g in range(GMT):
            lo = np.searchsorted(gt, g)
            hi = np.searchsorted(gt, g + 1)
            segs_g.append(src_l[lo:hi].astype(np.int16))
            segs_l.append((dst[lo:hi] - g * P).astype(np.float32))
        C_g.append(segs_g)
        C_l.append(segs_l)
    Cg_str, Cl_str, C_seg_chunks = _segment_streams(C_g, C_l, W)
    S["C_chunks"] = np.array(C_seg_chunks)          # [GMT]
    S["NC"] = int(S["C_chunks"].sum())

    # ---- Pass D streams: labels by user core, segments = movie range ----
    l_core = lbl_user // USR
    D_u, D_m, D_pos = [], [], []
    for c in range(W):
        m = l_core == c
        idxs = np.nonzero(m)[0]
        ul = (lbl_user[m] - c * USR)
        mv = lbl_movie[m]
        rng = np.minimum(mv // 27008, 2)
        order = np.argsort(rng, kind="stable")
        segs_u, segs_m = [], []
        for r in range(NRNG):
            mm = rng[order] == r
            segs_u.append(ul[order][mm].astype(np.int16))
            segs_m.append((mv[order][mm] - RNG_STARTS[r]).astype(np.int16))
        D_u.append(segs_u)
        D_m.append(segs_m)
        D_pos.append(idxs[order])       # original label index per real stream slot
    # pad segments to max-over-cores
    D_seg_chunks = []
    for r in range(NRNG):
        mx = max(len(D_u[c][r]) for c in range(W))
        D_seg_chunks.append((mx + P - 1) // P)
    S["D_chunks"] = np.array(D_seg_chunks)
    S["ND"] = int(S["D_chunks"].sum())
    Du_str, Dm_str, D_real = [], [], []
    for c in range(W):
        du = np.zeros(S["ND"] * P, np.int16)
        dm = np.zeros(S["ND"] * P, np.int16)
        real = np.full(S["ND"] * P, -1, np.int64)
        pos = 0
        k = 0
        for r in range(NRNG):
            n = len(D_u[c][r])
            du[pos: pos + n] = D_u[c][r]
            dm[pos: pos + n] = D_m[c][r]
            real[pos: pos + n] = D_pos[c][k: k + n]
            k += n
            pos += D_seg_chunks[r] * P
        Du_str.append(du)
        Dm_str.append(dm)
        D_real.append(real)

    iota_rep = np.tile(np.arange(P, dtype=np.float32)[None, :], (P, 4))

    per_core = []
    for c in range(W):
        per_core.append({
            "b_loc": Bl_str[c],
            "b_gidx": _wrap16(Bg_str[c]),
            "c_loc": Cl_str[c],
            "c_gidx": _wrap16(Cg_str[c]),
            "d_uidx": _wrap16(Du_str[c]),
            "d_midx": _wrap16(Dm_str[c]),
            "iota": iota_rep,
        })
    return S, per_core, D_real


def _gather_groups(n_chunks_list):
    """Split a list of per-segment chunk counts into dma_gather groups of <=GROUP chunks,
    never crossing the segment-list boundary. Returns list of group sizes (in chunks)."""
    total = sum(n_chunks_list)
    groups = []
    rem = total
    while rem > 0:
        g = min(GROUP, rem)
        groups.append(g)
        rem -= g
    return groups


# ---------------- device program ----------------

def build_program(S):
    import os
    UPTO = int(os.environ.get('KUPTO', '9'))
    nc = bacc.Bacc("TRN2", target_bir_lowering=False, debug=False, num_devices=W)
    NB, NC, ND = S["NB"], S["NC"], S["ND"]
    B_chunks, C_chunks, D_chunks = S["B_chunks"], S["C_chunks"], S["D_chunks"]

    # ---- kernel I/O ----
    featsT = nc.dram_tensor("featsT", [FD, MSL], f32, kind="ExternalInput")
    wm = nc.dram_tensor("wm", [FD, H], f32, kind="ExternalInput")
    u0 = nc.dram_tensor("u0", [H], f32, kind="ExternalInput")
    wnames = ["bm", "wl1_um", "bl1_um", "wr1_um", "wl1_mu", "bl1_mu", "wr1_mu",
              "wl2_um", "bl2_um", "wr2_um", "wl2_mu", "bl2_mu", "wr2_mu"]
    wt = {}
    for n in wnames:
        shape = [H] if n.startswith("b") else [H, H]
        wt[n] = nc.dram_tensor(n, shape, f32, kind="ExternalInput")
    iota_in = nc.dram_tensor("iota", [P, 4 * P], f32, kind="ExternalInput")
    b_loc = nc.dram_tensor("b_loc", [P, NB], f32, kind="ExternalInput")
    b_gidx = nc.dram_tensor("b_gidx", [P, NB * 8], i16, kind="ExternalInput")
    c_loc = nc.dram_tensor("c_loc", [P, NC], f32, kind="ExternalInput")
    c_gidx = nc.dram_tensor("c_gidx", [P, NC * 8], i16, kind="ExternalInput")
    d_uidx = nc.dram_tensor("d_uidx", [P, ND * 8], i16, kind="ExternalInput")
    d_midx = nc.dram_tensor("d_midx", [P, ND * 8], i16, kind="ExternalInput")
    out = nc.dram_tensor("out", [P, ND], f32, kind="ExternalOutput")

    # ---- internal DRAM ----
    cnt_local = nc.dram_tensor("cnt_local", [GMT, P], f32)          # strip rows = tiles
    cnt_rs = nc.dram_tensor("cnt_rs", [MT, P], f32)
    xcat_slice = nc.dram_tensor("xcat_slice", [MSL, 2 * H], bf16)
    xcat_full = nc.dram_tensor("xcat_full", [NMP, 2 * H], bf16, addr_space="Shared")
    mht_stash = nc.dram_tensor("mht_stash", [P, MSL], bf16)
    userh = nc.dram_tensor("userh", [USR, H], bf16)
    usero = nc.dram_tensor("usero", [USR, H], bf16)
    partials = nc.dram_tensor("partials", [NMP, H], bf16)
    parts_recv = nc.dram_tensor("parts_recv", [NMP, H], bf16)
    mo_slice = nc.dram_tensor("mo_slice", [MSL, H], bf16)
    mo_full = nc.dram_tensor("mo_full", [NMP, H], bf16, addr_space="Shared")

    rg = [list(range(W))]

    from contextlib import ExitStack
    with tile.TileContext(nc) as tc, ExitStack() as stack:
        cst = stack.enter_context(tc.tile_pool(name="cst", bufs=1))

        # ---------- constants ----------
        iota_t = cst.tile([P, 4 * P], f32)
        nc.sync.dma_start(out=iota_t[:], in_=iota_in[:])
        ones_bf = cst.tile([P, 1], bf16)
        nc.vector.memset(ones_bf[:], 1.0)
        ident_bf = cst.tile([P, P], bf16)
        make_identity(nc, ident_bf[:])
        ones_row = cst.tile([1, P], f32)
        nc.vector.memset(ones_row[:], 1.0)
        u0_col = cst.tile([P, 1], f32)
        nc.sync.dma_start(out=u0_col[:], in_=u0[:, None])

        wtile = {}
        for n in wnames:
            if n.startswith("b"):
                t = cst.tile([1, P], f32, tag=f"w_{n}")
                nc.sync.dma_start(out=t[:], in_=wt[n][None, :])
            else:
                t = cst.tile([P, P], f32, tag=f"w_{n}")
                nc.sync.dma_start(out=t[:], in_=wt[n][:])
            wtile[n] = t
        # casts
        w_r = {}
        for n in ["wr1_um", "wl1_mu"]:
            t = cst.tile([P, P], f32r, tag=f"wr_{n}")
            nc.vector.tensor_copy(out=t[:], in_=wtile[n][:])
            w_r[n] = t
        w_bf = {}
        for n in ["wr2_mu", "wr2_um", "wl2_um", "wl2_mu"]:
            t = cst.tile([P, P], bf16, tag=f"wbf_{n}")
            nc.vector.tensor_copy(out=t[:], in_=wtile[n][:])
            w_bf[n] = t
        wm_r = []
        for k in range(4):
            t = cst.tile([P, H], f32r, tag=f"wm_{k}")
            nc.sync.dma_start(out=t[:], in_=wm[k * P:(k + 1) * P, :].bitcast(f32r))
            wm_r.append(t)
        bm_col = cst.tile([P, 1], f32)
        nc.sync.dma_start(out=bm_col[:], in_=wt["bm"][:, None])
        bl1um_col = cst.tile([P, 1], f32)
        nc.sync.dma_start(out=bl1um_col[:], in_=wt["bl1_um"][:, None])

        # v_row = u0 @ Wl1_um   [1,128]; r1_row = u0 @ Wr1_mu
        with tc.tile_pool(name="psc", bufs=2, space="PSUM") as psc:
            vp = psc.tile([1, P], f32, space="PSUM", tag="vrow")
            nc.tensor.matmul(out=vp[:], lhsT=u0_col[:], rhs=wtile["wl1_um"][:], start=True, stop=True)
            v_row = cst.tile([1, P], f32)
            nc.vector.tensor_copy(out=v_row[:], in_=vp[:])
            rp = psc.tile([1, P], f32, space="PSUM", tag="vrow")
            nc.tensor.matmul(out=rp[:], lhsT=u0_col[:], rhs=wtile["wr1_mu"][:], start=True, stop=True)
            b1_row = cst.tile([1, P], f32)
            nc.vector.tensor_tensor(out=b1_row[:], in0=rp[:], in1=wtile["bl1_mu"][:], op=mybir.AluOpType.add)

            def bcast_row(row_ap, tag):
                ps = psc.tile([P, P], f32, space="PSUM", tag="bcast")
                nc.tensor.matmul(out=ps[:], lhsT=ones_row[:], rhs=row_ap, start=True, stop=True)
                t = cst.tile([P, P], f32, tag=tag)
                nc.vector.tensor_copy(out=t[:], in_=ps[:])
                return t

            Vcast = bcast_row(v_row[:], "Vcast")
            B1cast = bcast_row(b1_row[:], "B1cast")
            B2cast = bcast_row(wtile["bl2_mu"][:], "B2cast")
            B3cast = bcast_row(wtile["bl2_um"][:], "B3cast")

        # ---------- Pass A: movie in-degree counts ----------
        with tc.tile_pool(name="pa_sb", bufs=1) as pa_sb, \
             tc.tile_pool(name="pa_s4", bufs=4) as pa_s4, \
             tc.tile_pool(name="pa_ps", bufs=4, space="PSUM") as pa_ps, \
             tc.tile_pool(name="pa_st", bufs=2) as pa_st:
            cloc_t = pa_sb.tile([P, NC], f32)
            nc.sync.dma_start(out=cloc_t[:], in_=c_loc[:])
            cntstrip = pa_sb.tile([P, GMT], f32)
            pos = 0
            for g in range(GMT):
                n = int(C_chunks[g])
                if n == 0:
                    nc.vector.memset(cntstrip[:, g:g + 1], 0.0)
                    continue
                ps = pa_ps.tile([P, 8], f32, space="PSUM", tag="cnt")
                for c4 in range((n + 3) // 4):
                    nch = min(4, n - c4 * 4)
                    s4 = pa_s4.tile([P, 4 * P], bf16, tag="s4")
                    nc.vector.tensor_tensor(
                        out=s4[:].rearrange("p (k n) -> p k n", k=4),
                        in0=iota_t[:].rearrange("p (k n) -> p k n", k=4),
                        in1=cloc_t[:, pos + c4 * 4: pos + c4 * 4 + 4][:, :, None].to_broadcast([P, 4, P]),
                        op=mybir.AluOpType.is_equal,
                    )
                    for j in range(nch):
                        ch = c4 * 4 + j
                        nc.tensor.matmul(
                            out=ps[:, 0:1],
                            lhsT=s4[:, j * P:(j + 1) * P],
                            rhs=ones_bf[:],
                            start=(ch == 0), stop=(ch == n - 1),
                        )
                nc.vector.tensor_copy(out=cntstrip[:, g:g + 1], in_=ps[:, 0:1])
                pos += n
            # strip [128, GMT] -> DRAM [GMT, 128] (transposed) then ReduceScatter
            st = pa_st.tile([P, GMT], f32)
            nc.vector.tensor_copy(out=st[:], in_=cntstrip[:])
            nc.sync.dma_start(out=cnt_local[:].transpose([1, 0]), in_=st[:])
        nc.gpsimd.collective_compute(
            "ReduceScatter", mybir.AluOpType.add, replica_groups=rg,
            ins=[cnt_local[:].opt()], outs=[cnt_rs[:].opt()])

        if UPTO >= 2:
            # ---------- Stage 0: movie-side tables ----------
            NCT = (MSL + 511) // 512     # 20 col-tiles (last = 384)
            with tc.tile_pool(name="s0_sb", bufs=1) as s0_sb, \
                 tc.tile_pool(name="s0_mx", bufs=2) as s0_mx, \
                 tc.tile_pool(name="s0_ft", bufs=3) as s0_ft, \
                 tc.tile_pool(name="s0_ps", bufs=1, space="PSUM") as s0_ps, \
                 tc.tile_pool(name="s0_pt", bufs=2, space="PSUM") as s0_pt, \
                 tc.tile_pool(name="s0_stg", bufs=3) as s0_stg:
                p1T = s0_sb.tile([P, MSL], bf16)
                A_fm = s0_sb.tile([P, MSL], bf16)
                mhT = s0_sb.tile([P, MSL], bf16)
                cntcols = s0_sb.tile([P, MT], f32)
                nc.sync.dma_start(out=cntcols[:], in_=cnt_rs[:].transpose([1, 0]))
                indcols = s0_sb.tile([P, MT], f32)
                nc.vector.tensor_scalar(
                    out=indcols[:], in0=cntcols[:], scalar1=0.0, scalar2=None,
                    op0=mybir.AluOpType.is_gt)

                for j in range(NCT):
                    c0 = j * 512
                    cw = min(512, MSL - c0)
                    mxps = s0_ps.tile([P, 512], f32, space="PSUM", tag="mx")
                    for k in range(4):
                        ft = s0_ft.tile([P, 512], f32r, tag="ft")
                        nc.sync.dma_start(out=ft[:, :cw], in_=featsT[k * P:(k + 1) * P, c0:c0 + cw].bitcast(f32r))
                        nc.tensor.matmul(out=mxps[:, :cw], lhsT=wm_r[k][:], rhs=ft[:, :cw],
                                         start=(k == 0), stop=(k == 3))
                    mxt = s0_mx.tile([P, 512], f32r, tag="mxt")
                    nc.vector.tensor_tensor(out=mxt[:, :cw], in0=mxps[:, :cw],
                                            in1=bm_col[:].to_broadcast([P, cw]),
                                            op=mybir.AluOpType.add)
                    p1ps = s0_ps.tile([P, 512], f32, space="PSUM", tag="p1")
                    nc.tensor.matmul(out=p1ps[:, :cw], lhsT=w_r["wl1_mu"][:], rhs=mxt[:, :cw],
                                     start=True, stop=True)
                    nc.vector.tensor_copy(out=p1T[:, c0:c0 + cw], in_=p1ps[:, :cw])
                    aps = s0_ps.tile([P, 512], f32, space="PSUM", tag="A")
                    nc.tensor.matmul(out=aps[:, :cw], lhsT=w_r["wr1_um"][:], rhs=mxt[:, :cw],
                                     start=True, stop=True)
                    nc.vector.tensor_tensor(out=A_fm[:, c0:c0 + cw], in0=aps[:, :cw],
                                            in1=bl1um_col[:].to_broadcast([P, cw]),
                                            op=mybir.AluOpType.add)

                # per 128-tile: movie_h row-major then back to feature-major
                for t in range(MT):
                    c0 = t * P
                    tp = s0_pt.tile([P, P], bf16, space="PSUM", tag="tp")
                    nc.tensor.transpose(out=tp[:], in_=A_fm[:, c0:c0 + P], identity=ident_bf[:])
                    term = s0_stg.tile([P, P], f32, tag="term")
                    nc.vector.tensor_tensor(out=term[:], in0=Vcast[:],
                                            in1=indcols[:, t:t + 1].to_broadcast([P, P]),
                                            op=mybir.AluOpType.mult)
                    mhrow = s0_stg.tile([P, P], bf16, tag="mhrow")
                    nc.vector.tensor_tensor(out=mhrow[:], in0=tp[:], in1=term[:],
                                            op=mybir.AluOpType.add)
                    nc.vector.tensor_scalar_max(out=mhrow[:], in0=mhrow[:], scalar1=0.0)
                    tp2 = s0_pt.tile([P, P], bf16, space="PSUM", tag="tp2")
                    nc.tensor.transpose(out=tp2[:], in_=mhrow[:], identity=ident_bf[:])
                    nc.vector.tensor_copy(out=mhT[:, c0:c0 + P], in_=tp2[:])
                nc.sync.dma_start(out=mht_stash[:], in_=mhT[:])

                # p2T = Wl2_mu.T @ mhT  (bf16)
                p2T = s0_sb.tile([P, MSL], bf16)
                for j in range(NCT):
                    c0 = j * 512
                    cw = min(512, MSL - c0)
                    ps = s0_ps.tile([P, 512], f32, space="PSUM", tag="p2")
                    nc.tensor.matmul(out=ps[:, :cw], lhsT=w_bf["wl2_mu"][:], rhs=mhT[:, c0:c0 + cw],
                                     start=True, stop=True)
                    nc.vector.tensor_copy(out=p2T[:, c0:c0 + cw], in_=ps[:, :cw])

                # transpose to row-major X_cat slice and store
                for t in range(MT):
                    c0 = t * P
                    stg = s0_stg.tile([P, 2 * H], bf16, tag="xrow")
                    tp = s0_pt.tile([P, P], bf16, space="PSUM", tag="tp")
                    nc.tensor.transpose(out=tp[:], in_=p1T[:, c0:c0 + P], identity=ident_bf[:])
                    nc.vector.tensor_copy(out=stg[:, 0:H], in_=tp[:])
                    tp2 = s0_pt.tile([P, P], bf16, space="PSUM", tag="tp2")
                    nc.tensor.transpose(out=tp2[:], in_=p2T[:, c0:c0 + P], identity=ident_bf[:])
                    nc.vector.tensor_copy(out=stg[:, H:2 * H], in_=tp2[:])
                    nc.sync.dma_start(out=xcat_slice[c0:c0 + P, :], in_=stg[:])

            nc.gpsimd.collective_compute(
                "AllGather", mybir.AluOpType.bypass, replica_groups=rg,
                ins=[xcat_slice[:].opt()], outs=[xcat_full[:].opt()])

        if UPTO >= 3:
            # ---------- Pass B: user-side fused aggregation ----------
            ACC = 257  # [p1sum 128 | p2sum 128 | cnt 1]
            with tc.tile_pool(name="pb_sb", bufs=1) as pb_sb, \
                 tc.tile_pool(name="pb_s4", bufs=4) as pb_s4, \
                 tc.tile_pool(name="pb_g", bufs=3) as pb_g, \
                 tc.tile_pool(name="pb_gi", bufs=3) as pb_gi, \
                 tc.tile_pool(name="pb_ps", bufs=2, space="PSUM") as pb_ps, \
                 tc.tile_pool(name="pb_pc", bufs=2, space="PSUM") as pb_pc, \
                 tc.tile_pool(name="pb_pt", bufs=3, space="PSUM") as pb_pt, \
                 tc.tile_pool(name="pb_stg", bufs=4) as pb_stg:
                bloc_t = pb_sb.tile([P, NB], f32)
                nc.sync.dma_start(out=bloc_t[:], in_=b_loc[:])
                accB = pb_sb.tile([P, UT * ACC], bf16)
                nc.vector.memset(accB[:], 0.0)

                pos = 0          # global chunk position (stream)
                for r in range(NRNG):
                    table = xcat_full[RNG_STARTS[r]:RNG_ENDS[r], :]
                    sub_chunks = int(B_chunks[r].sum())
                    # gather groups for this sub-pass
                    gpos = 0
                    gbufs = []
                    while gpos < sub_chunks:
                        gn = min(GROUP, sub_chunks - gpos)
                        gb = pb_g.tile([P, GROUP * 2 * H], bf16, tag="gbuf")
                        gi = pb_gi.tile([P, GROUP * 8], i16, tag="gidx")
                        col0 = (pos + gpos) * 8
                        nc.sync.dma_start(out=gi[:, :gn * 8], in_=b_gidx[:, col0: col0 + gn * 8])
                        nc.gpsimd.dma_gather(
                            out_ap=gb[:, :gn * 2 * H].rearrange("p (c n) -> p c n", c=gn),
                            in_ap=table,
                            idxs_ap=gi[:, :gn * 8],
                            num_idxs=gn * P,
                            num_idxs_reg=gn * P,
                            elem_size=2 * H,
                        )
                        gbufs.append((gpos, gn, gb))
                        gpos += gn

                    def get_slot(sub_pos):
                        for g0, gn, gb in gbufs:
                            if g0 <= sub_pos < g0 + gn:
                                return gb, sub_pos - g0
                        raise AssertionError

                    sub_pos = 0
                    for t in range(UT):
                        n = int(B_chunks[r][t])
                        if n == 0:
                            continue
                        ps = pb_ps.tile([P, 2 * H], f32, space="PSUM", tag="ps")
                        pc = pb_pc.tile([P, 8], f32, space="PSUM", tag="pc")
                        for c4 in range((n + 3) // 4):
                            nch = min(4, n - c4 * 4)
                            s4 = pb_s4.tile([P, 4 * P], bf16, tag="s4")
                            cc = pos + sub_pos + c4 * 4
                            nc.vector.tensor_tensor(
                                out=s4[:].rearrange("p (k n) -> p k n", k=4),
                                in0=iota_t[:].rearrange("p (k n) -> p k n", k=4),
                                in1=bloc_t[:, cc: cc + 4][:, :, None].to_broadcast([P, 4, P]),
                                op=mybir.AluOpType.is_equal,
                            )
                            for j in range(nch):
                                ch = c4 * 4 + j
                                gb, slot = get_slot(sub_pos + ch)
                                nc.tensor.matmul(
                                    out=ps[:],
                                    lhsT=s4[:, j * P:(j + 1) * P],
                                    rhs=gb[:, slot * 2 * H:(slot + 1) * 2 * H],
                                    start=(ch == 0), stop=(ch == n - 1),
                                )
                                nc.tensor.matmul(
                                    out=pc[:, 0:1],
                                    lhsT=s4[:, j * P:(j + 1) * P],
                                    rhs=ones_bf[:],
                                    start=(ch == 0), stop=(ch == n - 1),
                                )
                        a0 = t * ACC
                        nc.vector.tensor_tensor(out=accB[:, a0:a0 + 2 * H], in0=ps[:],
                                                in1=accB[:, a0:a0 + 2 * H], op=mybir.AluOpType.add)
                        nc.vector.tensor_tensor(out=accB[:, a0 + 2 * H:a0 + ACC], in0=pc[:, 0:1],
                                                in1=accB[:, a0 + 2 * H:a0 + ACC], op=mybir.AluOpType.add)
                        sub_pos += n
                    pos += sub_chunks

                # ---- epilogue: user_h / user_o per tile ----
                cntv = pb_stg.tile([P, UT], f32, tag="cntv")
                nc.vector.tensor_copy(
                    out=cntv[:],
                    in_=accB[:].rearrange("p (t a) -> p t a", a=ACC)[:, :, 2 * H:2 * H + 1].squeeze(2))
                nc.vector.tensor_scalar_max(out=cntv[:], in0=cntv[:], scalar1=1.0)
                recipv = pb_stg.tile([P, UT], f32, tag="recipv")
                nc.vector.reciprocal(out=recipv[:], in_=cntv[:])

                for t in range(UT):
                    a0 = t * ACC
                    rc = recipv[:, t:t + 1]
                    uh = pb_stg.tile([P, H], bf16, tag="uh")
                    nc.vector.tensor_tensor(out=uh[:], in0=accB[:, a0:a0 + H],
                                            in1=rc.to_broadcast([P, H]), op=mybir.AluOpType.mult)
                    nc.vector.tensor_tensor(out=uh[:], in0=uh[:], in1=B1cast[:],
                                            op=mybir.AluOpType.add)
                    nc.vector.tensor_scalar_max(out=uh[:], in0=uh[:], scalar1=0.0)
                    tp = pb_pt.tile([P, P], bf16, space="PSUM", tag="ep")
                    nc.tensor.transpose(out=tp[:], in_=uh[:], identity=ident_bf[:])
                    uht = pb_stg.tile([P, P], bf16, tag="uhts")
                    nc.vector.tensor_copy(out=uht[:], in_=tp[:])
                    # Pass C gather table = user_h @ Wl2_um (pre-multiplied; linearity)
                    t2ps = pb_pt.tile([P, P], f32, space="PSUM", tag="ep")
                    nc.tensor.matmul(out=t2ps[:], lhsT=uht[:], rhs=w_bf["wl2_um"][:],
                                     start=True, stop=True)
                    uh2 = pb_stg.tile([P, P], bf16, tag="uh2")
                    nc.vector.tensor_copy(out=uh2[:], in_=t2ps[:])
                    nc.sync.dma_start(out=userh[t * P:(t + 1) * P, :], in_=uh2[:])
                    # user_o = p2sum*recip + B2cast + uh @ Wr2_mu
                    rps = pb_pt.tile([P, P], f32, space="PSUM", tag="ep")
                    nc.tensor.matmul(out=rps[:], lhsT=uht[:], rhs=w_bf["wr2_mu"][:],
                                     start=True, stop=True)
                    uo = pb_stg.tile([P, H], f32, tag="uo")
                    nc.vector.tensor_tensor(out=uo[:], in0=accB[:, a0 + H:a0 + 2 * H],
                                            in1=rc.to_broadcast([P, H]), op=mybir.AluOpType.mult)
                    nc.vector.tensor_tensor(out=uo[:], in0=uo[:], in1=B2cast[:],
                                            op=mybir.AluOpType.add)
                    uo_bf = pb_stg.tile([P, H], bf16, tag="uobf")
                    nc.vector.tensor_tensor(out=uo_bf[:], in0=uo[:], in1=rps[:],
                                            op=mybir.AluOpType.add)
                    nc.sync.dma_start(out=usero[t * P:(t + 1) * P, :], in_=uo_bf[:])

        if UPTO >= 4:
            # ---------- Pass C: movie-side aggregation of user_h ----------
            with tc.tile_pool(name="pc_sb", bufs=1) as pc_sb, \
                 tc.tile_pool(name="pc_s4", bufs=4) as pc_s4, \
                 tc.tile_pool(name="pc_g", bufs=3) as pc_g, \
                 tc.tile_pool(name="pc_gi", bufs=3) as pc_gi, \
                 tc.tile_pool(name="pc_ps", bufs=2, space="PSUM") as pc_ps, \
                 tc.tile_pool(name="pc_stg", bufs=4) as pc_stg:
                cloc_t = pc_sb.tile([P, NC], f32)
                nc.sync.dma_start(out=cloc_t[:], in_=c_loc[:])

                gpos = 0
                gbufs = []
                while gpos < NC:
                    gn = min(GROUP, NC - gpos)
                    gb = pc_g.tile([P, GROUP * H], bf16, tag="gbuf")
                    gi = pc_gi.tile([P, GROUP * 8], i16, tag="gidx")
                    nc.sync.dma_start(out=gi[:, :gn * 8], in_=c_gidx[:, gpos * 8: (gpos + gn) * 8])
                    nc.gpsimd.dma_gather(
                        out_ap=gb[:, :gn * H].rearrange("p (c n) -> p c n", c=gn),
                        in_ap=userh[:],
                        idxs_ap=gi[:, :gn * 8],
                        num_idxs=gn * P,
                        num_idxs_reg=gn * P,
                        elem_size=H,
                    )
                    gbufs.append((gpos, gn, gb))
                    gpos += gn

                def get_slotC(p_):
                    for g0, gn, gb in gbufs:
                        if g0 <= p_ < g0 + gn:
                            return gb, p_ - g0
                    raise AssertionError

                pos = 0
                for g in range(GMT):
                    n = int(C_chunks[g])
                    if n == 0:
                        stg = pc_stg.tile([P, H], bf16, tag="pstg")
                        nc.vector.memset(stg[:], 0.0)
                        nc.sync.dma_start(out=partials[g * P:(g + 1) * P, :], in_=stg[:])
                        continue
                    ps = pc_ps.tile([P, H], f32, space="PSUM", tag="ps")
                    for c4 in range((n + 3) // 4):
                        nch = min(4, n - c4 * 4)
                        s4 = pc_s4.tile([P, 4 * P], bf16, tag="s4")
                        cc = pos + c4 * 4
                        nc.vector.tensor_tensor(
                            out=s4[:].rearrange("p (k n) -> p k n", k=4),
                            in0=iota_t[:].rearrange("p (k n) -> p k n", k=4),
                            in1=cloc_t[:, cc: cc + 4][:, :, None].to_broadcast([P, 4, P]),
                            op=mybir.AluOpType.is_equal,
                        )
                        for j in range(nch):
                            ch = c4 * 4 + j
                            gb, slot = get_slotC(pos + ch)
                            nc.tensor.matmul(
                                out=ps[:],
                                lhsT=s4[:, j * P:(j + 1) * P],
                                rhs=gb[:, slot * H:(slot + 1) * H],
                                start=(ch == 0), stop=(ch == n - 1),
                            )
                    stg = pc_stg.tile([P, H], bf16, tag="pstg")
                    nc.vector.tensor_copy(out=stg[:], in_=ps[:])
                    nc.sync.dma_start(out=partials[g * P:(g + 1) * P, :], in_=stg[:])
                    pos += n

            nc.gpsimd.collective_compute(
                "AllToAll", mybir.AluOpType.bypass, replica_groups=rg,
                ins=[partials[:].opt()], outs=[parts_recv[:].opt()])

        if UPTO >= 5:
            # ---------- movie_o ----------
            with tc.tile_pool(name="mo_sb", bufs=1) as mo_sb, \
                 tc.tile_pool(name="mo_in", bufs=3) as mo_in, \
                 tc.tile_pool(name="mo_ps", bufs=2, space="PSUM") as mo_ps, \
                 tc.tile_pool(name="mo_stg", bufs=4) as mo_stg:
                cntcols = mo_sb.tile([P, MT], f32)
                nc.sync.dma_start(out=cntcols[:], in_=cnt_rs[:].transpose([1, 0]))
                nc.vector.tensor_scalar_max(out=cntcols[:], in0=cntcols[:], scalar1=1.0)
                recipm = mo_sb.tile([P, MT], f32)
                nc.vector.reciprocal(out=recipm[:], in_=cntcols[:])

                rv = parts_recv[:].rearrange("(s m) h -> s m h", s=W)
                for t in range(MT):
                    pin = mo_in.tile([P, W * H], bf16, tag="pin")
                    nc.sync.dma_start(
                        out=pin[:].rearrange("p (s h) -> p s h", s=W),
                        in_=rv[:, t * P:(t + 1) * P, :].transpose([1, 0, 2]))
                    s1 = mo_stg.tile([P, 4 * H], f32, tag="s1")
                    nc.vector.tensor_tensor(out=s1[:], in0=pin[:, 0:4 * H],
                                            in1=pin[:, 4 * H:8 * H], op=mybir.AluOpType.add)
                    s2 = mo_stg.tile([P, 2 * H], f32, tag="s2")
                    nc.vector.tensor_tensor(out=s2[:], in0=s1[:, 0:2 * H],
                                            in1=s1[:, 2 * H:4 * H], op=mybir.AluOpType.add)
                    s3 = mo_stg.tile([P, H], f32, tag="s3")
                    nc.vector.tensor_tensor(out=s3[:], in0=s2[:, 0:H],
                                            in1=s2[:, H:2 * H], op=mybir.AluOpType.add)
                    # root term
                    mh = mo_in.tile([P, P], bf16, tag="mh")
                    nc.sync.dma_start(out=mh[:], in_=mht_stash[:, t * P:(t + 1) * P])
                    rps = mo_ps.tile([P, P], f32, space="PSUM", tag="mroot")
                    nc.tensor.matmul(out=rps[:], lhsT=mh[:], rhs=w_bf["wr2_um"][:],
                                     start=True, stop=True)
                    mo_t = mo_stg.tile([P, H], f32, tag="mo1")
                    nc.vector.tensor_tensor(out=mo_t[:], in0=s3[:],
                                            in1=recipm[:, t:t + 1].to_broadcast([P, H]),
                                            op=mybir.AluOpType.mult)
                    nc.vector.tensor_tensor(out=mo_t[:], in0=mo_t[:], in1=B3cast[:],
                                            op=mybir.AluOpType.add)
                    mo_bf = mo_stg.tile([P, H], bf16, tag="mo2")
                    nc.vector.tensor_tensor(out=mo_bf[:], in0=mo_t[:], in1=rps[:],
                                            op=mybir.AluOpType.add)
                    nc.sync.dma_start(out=mo_slice[t * P:(t + 1) * P, :], in_=mo_bf[:])

            nc.gpsimd.collective_compute(
                "AllGather", mybir.AluOpType.bypass, replica_groups=rg,
                ins=[mo_slice[:].opt()], outs=[mo_full[:].opt()])

        if UPTO >= 6:
            # ---------- Pass D: label dots ----------
            with tc.tile_pool(name="pd_sb", bufs=1) as pd_sb, \
                 tc.tile_pool(name="pd_g", bufs=4) as pd_g, \
                 tc.tile_pool(name="pd_gi", bufs=4) as pd_gi, \
                 tc.tile_pool(name="pd_stg", bufs=4) as pd_stg:
                outstrip = pd_sb.tile([P, ND], f32)
                pos = 0
                for r in range(NRNG):
                    n_r = int(D_chunks[r])
                    table = mo_full[RNG_STARTS[r]:RNG_ENDS[r], :]
                    gpos = 0
                    while gpos < n_r:
                        gn = min(GROUP, n_r - gpos)
                        gu = pd_g.tile([P, GROUP * H], bf16, tag="gu")
                        gm = pd_g.tile([P, GROUP * H], bf16, tag="gm")
                        giu = pd_gi.tile([P, GROUP * 8], i16, tag="giu")
                        gim = pd_gi.tile([P, GROUP * 8], i16, tag="gim")
                        col0 = (pos + gpos) * 8
                        nc.sync.dma_start(out=giu[:, :gn * 8], in_=d_uidx[:, col0: col0 + gn * 8])
                        nc.sync.dma_start(out=gim[:, :gn * 8], in_=d_midx[:, col0: col0 + gn * 8])
                        nc.gpsimd.dma_gather(
                            out_ap=gu[:, :gn * H].rearrange("p (c n) -> p c n", c=gn),
                            in_ap=usero[:], idxs_ap=giu[:, :gn * 8],
                            num_idxs=gn * P, num_idxs_reg=gn * P, elem_size=H)
                        nc.gpsimd.dma_gather(
                            out_ap=gm[:, :gn * H].rearrange("p (c n) -> p c n", c=gn),
                            in_ap=table, idxs_ap=gim[:, :gn * 8],
                            num_idxs=gn * P, num_idxs_reg=gn * P, elem_size=H)
                        for s in range(gn):
                            pr = pd_stg.tile([P, H], f32, tag="pr")
                            nc.vector.tensor_tensor(out=pr[:], in0=gu[:, s * H:(s + 1) * H],
                                                    in1=gm[:, s * H:(s + 1) * H],
                                                    op=mybir.AluOpType.mult)
                            ch = pos + gpos + s
                            nc.vector.tensor_reduce(
                                out=outstrip[:, ch:ch + 1], in_=pr[:],
                                axis=mybir.AxisListType.X, op=mybir.AluOpType.add)
                        gpos += gn
                    pos += n_r
                nc.sync.dma_start(out=out[:], in_=outstrip[:])
        else:
            with tc.tile_pool(name="dummy", bufs=1) as dp:
                z = dp.tile([P, ND], f32)
                nc.vector.memset(z[:], 0.0)
                nc.sync.dma_start(out=out[:], in_=z[:])

    nc.compile()
    return nc


# ---------------- entry point ----------------

_CACHE = {}
TRACE = False
LAST_EXEC_NS = None
LAST_RESULTS = None


def kernel(movie_feats, user_init, edge_src, edge_dst, lbl_user, lbl_movie, n_users,
           Wm, bm,
           Wl1_um, bl1_um, Wr1_um, Wl1_mu, bl1_mu, Wr1_mu,
           Wl2_um, bl2_um, Wr2_um, Wl2_mu, bl2_mu, Wr2_mu):
    movie_feats = np.asarray(movie_feats, dtype=np.float32)
    S, per_core, D_real = preprocess(edge_src, edge_dst, lbl_user, lbl_movie)

    key = (S["NB"], S["NC"], S["ND"],
           S["B_chunks"].tobytes(), S["C_chunks"].tobytes(), S["D_chunks"].tobytes())
    if key in _CACHE:
        nc = _CACHE[key]
    else:
        nc = build_program(S)
        _CACHE[key] = nc

    featsT = np.zeros((FD, NMP), np.float32)
    featsT[:, :NM] = movie_feats.T

    weights = {
        "wm": np.asarray(Wm, np.float32), "u0": np.asarray(user_init, np.float32),
        "bm": np.asarray(bm, np.float32),
        "wl1_um": np.asarray(Wl1_um, np.float32), "bl1_um": np.asarray(bl1_um, np.float32),
        "wr1_um": np.asarray(Wr1_um, np.float32),
        "wl1_mu": np.asarray(Wl1_mu, np.float32), "bl1_mu": np.asarray(bl1_mu, np.float32),
        "wr1_mu": np.asarray(Wr1_mu, np.float32),
        "wl2_um": np.asarray(Wl2_um, np.float32), "bl2_um": np.asarray(bl2_um, np.float32),
        "wr2_um": np.asarray(Wr2_um, np.float32),
        "wl2_mu": np.asarray(Wl2_mu, np.float32), "bl2_mu": np.asarray(bl2_mu, np.float32),
        "wr2_mu": np.asarray(Wr2_mu, np.float32),
    }

    in_maps = []
    for c in range(W):
        m = {"featsT": np.ascontiguousarray(featsT[:, c * MSL:(c + 1) * MSL])}
        m.update(weights)
        pc = per_core[c]
        m.update({
            "iota": pc["iota"],
            "b_loc": pc["b_loc"], "b_gidx": pc["b_gidx"],
            "c_loc": pc["c_loc"], "c_gidx": pc["c_gidx"],
            "d_uidx": pc["d_uidx"], "d_midx": pc["d_midx"],
        })
        in_maps.append(m)

    global LAST_EXEC_NS, LAST_RESULTS
    res = run_bass_kernel_spmd(nc, in_maps, core_ids=list(range(W)), trace=TRACE)
    LAST_EXEC_NS = res.exec_time_ns
    LAST_RESULTS = res

    EL = len(np.asarray(lbl_user))
    out_full = np.zeros(EL, np.float32)
    for c in range(W):
        vals = res.results[c]["out"].T.reshape(-1)       # stream order
        real = D_real[c]
        mask = real >= 0
        out_full[real[mask]] = vals[mask]
    return out_full



# revision 12
# speedup vs baseline: 1.4313x; 1.4313x over previous
"""Trainium2 Bass kernel for nn_NovaLinkPredictor (hetero GraphSAGE link predictor), v2.

8-core SPMD strategy (edge-parallel by user range):
  - Users sharded by range: 8 x 25088 rows. Movies: 8 x 10112 (padded 80896 global).
  - Stage 0: each core builds movie tables for its movie slice:
      xcat = [p1 | p2], p1 = movie_x@Wl1_mu + fold1, p2 = movie_h@Wl2_mu + bl2_mu,
      movie_h = relu(movie_x@Wr1_um + bl1_um + ind*(u0@Wl1_um)), movie_x = feats@Wm + bm.
      Biases/u0-terms folded into tables via rank-1 matmuls; AllGather xcat (bf16).
  - Pass B: per-edge gather of xcat rows (dst) + one-hot scatter matmuls into per-user-tile
    PSUM accs; segments (range r, user tile t) r-major; SBUF accumulator across ranges.
    Gathers use prepare_only SWDGE preps + trigger_dma; consumers carry explicit
    DMA-completion sem waits (Tile only wires prep-completion by itself).
  - Mean divisions use host-precomputed reciprocal degree vectors (np.bincount).
  - Pass B epilogue: user_h via scalar-engine relu(scale), userh table = user_h@Wl2_um+bl2_um,
    user_o = p2mean + user_h@Wr2_mu.
  - Pass C: gather userh rows (src) per dst-sorted edges, scatter into per-movie-tile partials,
    AllToAll, reduce 8 partial slices + root term (movie_hT resident) -> movie_o, AllGather.
  - Pass D: labels sorted by movie tile; user_o rows via transposed gather (feature-major),
    movie_o rows via one-hot select matmul from streamed mo tiles; dot via elementwise mult +
    ones-matmul partition reduce.
"""
import sys
sys.path.insert(0, "/opt/trn_rl_repo")
import numpy as np
import ml_dtypes

from concourse import bass, mybir, bacc, tile
from concourse.bass_utils import run_bass_kernel_spmd
from concourse.masks import make_identity

# ---------------- constants ----------------
H = 128
NU = 200000
NM = 80000
FD = 512
W = 8
P = 128

USR = 25088            # user rows per core (196 tiles)
UT = 196
NUP = USR * W          # 200704
MSL = 10112            # movie rows per core (79 tiles)
MT = 79
NMP = MSL * W          # 80896
GMT = NMP // P         # 632 global movie tiles

RNG_STARTS = [0, 27008, 54016]          # movie gather ranges (int16-safe)
RNG_ENDS = [27008, 54016, NMP]
NRNG = 3
GROUP = 8             # chunks per dma_gather (8*128 = 1024 rows)
SENT = 200.0          # one-hot sentinel (outside 0..127)

bf16 = mybir.dt.bfloat16
f32 = mybir.dt.float32
i16 = mybir.dt.int16
npbf16 = ml_dtypes.bfloat16


# ---------------- host-side preprocessing ----------------

def _wrap16(idx):
    """int16 stream -> [128, n/16] wrapped layout for dma_gather idxs."""
    n = idx.shape[0]
    assert n % 16 == 0
    w = idx.reshape(n // 16, 16).T.astype(np.int16)      # [16, n/16]
    return np.ascontiguousarray(np.tile(w, (8, 1)))      # [128, n/16]


def _segment_streams(gidx_list, loc_list, n_cores):
    """Pad each segment to the max-over-cores chunk count; return per-core
    (gidx_stream, loc_stream[128, NB] bf16) plus per-segment chunk counts."""
    nseg = len(gidx_list[0])
    seg_chunks = []
    for s in range(nseg):
        mx = max(len(gidx_list[c][s]) for c in range(n_cores))
        seg_chunks.append((mx + P - 1) // P)
    nb = sum(seg_chunks)
    g_streams, l_streams = [], []
    for c in range(n_cores):
        g = np.zeros(nb * P, np.int16)
        l = np.full(nb * P, SENT, np.float32)
        pos = 0
        for s in range(nseg):
            n = len(gidx_list[c][s])
            g[pos: pos + n] = gidx_list[c][s]
            l[pos: pos + n] = loc_list[c][s]
            pos += seg_chunks[s] * P
        g_streams.append(g)
        l_streams.append(np.ascontiguousarray(l.reshape(nb, P).T.astype(npbf16)))
    return g_streams, l_streams, seg_chunks


def preprocess(edge_src, edge_dst, lbl_user, lbl_movie):
    S = {}
    edge_src = np.asarray(edge_src).astype(np.int64)
    edge_dst = np.asarray(edge_dst).astype(np.int64)
    lbl_user = np.asarray(lbl_user).astype(np.int64)
    lbl_movie = np.asarray(lbl_movie).astype(np.int64)

    u_core = edge_src // USR
    u_loc = edge_src - u_core * USR

    # ---- Pass B streams: segments = (range r, user tile t), r-major ----
    B_g, B_l = [], []
    for c in range(W):
        m = u_core == c
        src_l = u_loc[m]
        dst = edge_dst[m]
        rng = np.minimum(dst // 27008, 2)
        tilev = src_l // P
        order = np.lexsort((dst, tilev, rng))
        src_l, dst, rng, tilev = src_l[order], dst[order], rng[order], tilev[order]
        segs_g, segs_l = [], []
        for r in range(NRNG):
            for t in range(UT):
                mm = (rng == r) & (tilev == t)
                segs_g.append((dst[mm] - RNG_STARTS[r]).astype(np.int16))
                segs_l.append((src_l[mm] - t * P).astype(np.float32))
        B_g.append(segs_g)
        B_l.append(segs_l)
    Bg_str, Bl_str, B_seg_chunks = _segment_streams(B_g, B_l, W)
    S["B_chunks"] = np.array(B_seg_chunks).reshape(NRNG, UT)
    S["NB"] = int(S["B_chunks"].sum())

    # ---- Pass C streams: segments = global movie tile g (dst-sorted) ----
    C_g, C_l = [], []
    for c in range(W):
        m = u_core == c
        src_l = u_loc[m]
        dst = edge_dst[m]
        order = np.argsort(dst, kind="stable")
        src_l, dst = src_l[order], dst[order]
        gt = dst // P
        segs_g, segs_l = [], []
        for g in range(GMT):
            lo = np.searchsorted(gt, g)
            hi = np.searchsorted(gt, g + 1)
            segs_g.append(src_l[lo:hi].astype(np.int16))
            segs_l.append((dst[lo:hi] - g * P).astype(np.float32))
        C_g.append(segs_g)
        C_l.append(segs_l)
    Cg_str, Cl_str, C_seg_chunks = _segment_streams(C_g, C_l, W)
    S["C_chunks"] = np.array(C_seg_chunks)
    S["NC"] = int(S["C_chunks"].sum())

    # ---- Pass D streams: labels by user core, sorted by movie tile ----
    l_core = lbl_user // USR
    D_u, D_ml, D_pos = [], [], []
    for c in range(W):
        m = l_core == c
        idxs = np.nonzero(m)[0]
        ul = lbl_user[m] - c * USR
        mv = lbl_movie[m]
        gl = mv // P
        order = np.argsort(gl, kind="stable")
        ul, mv, gl = ul[order], mv[order], gl[order]
        segs_u, segs_ml = [], []
        for g in range(GMT):
            lo = np.searchsorted(gl, g)
            hi = np.searchsorted(gl, g + 1)
            segs_u.append(ul[lo:hi].astype(np.int16))
            segs_ml.append((mv[lo:hi] - g * P).astype(np.float32))
        D_u.append(segs_u)
        D_ml.append(segs_ml)
        D_pos.append(idxs[order])
    D_seg_chunks = []
    for g in range(GMT):
        mx = max(len(D_u[c][g]) for c in range(W))
        D_seg_chunks.append((mx + P - 1) // P)
    S["D_chunks"] = np.array(D_seg_chunks)          # [GMT], mostly 1
    S["NDCH"] = int(S["D_chunks"].sum())
    NDCH = S["NDCH"]
    Du_str, Dml_str, D_real = [], [], []
    for c in range(W):
        du = np.zeros(NDCH * P, np.int16)
        dml = np.full(NDCH * P, SENT, np.float32)
        real = np.full(NDCH * P, -1, np.int64)
        pos = 0
        k = 0
        for g in range(GMT):
            n = len(D_u[c][g])
            du[pos: pos + n] = D_u[c][g]
            dml[pos: pos + n] = D_ml[c][g]
            real[pos: pos + n] = D_pos[c][k: k + n]
            k += n
            pos += D_seg_chunks[g] * P
        Du_str.append(du)
        Dml_str.append(dml.astype(npbf16)[None, :])     # [1, NDCH*128]
        D_real.append(real)

    # ---- host reciprocal degrees ----
    cnt_u = np.bincount(edge_src, minlength=NUP).astype(np.float64)
    recip_u = (1.0 / np.maximum(cnt_u, 1.0)).astype(np.float32)
    cnt_m = np.bincount(edge_dst, minlength=NMP).astype(np.float64)
    recip_m = (1.0 / np.maximum(cnt_m, 1.0)).astype(np.float32)
    ind_m = (cnt_m > 0).astype(npbf16)

    iota4 = np.tile(np.arange(P, dtype=np.float32)[None, :], (P, 4)).astype(npbf16)
    iota_col = np.arange(P, dtype=np.float32)[:, None].astype(npbf16)

    per_core = []
    for c in range(W):
        per_core.append({
            "b_loc": Bl_str[c],
            "b_gidx": _wrap16(Bg_str[c]),
            "c_loc": Cl_str[c],
            "c_gidx": _wrap16(Cg_str[c]),
            "d_uidx": _wrap16(Du_str[c]),
            "d_mloc": np.ascontiguousarray(Dml_str[c]),
            "iota4": iota4,
            "iota_col": iota_col,
            "recipu": np.ascontiguousarray(
                recip_u[c * USR:(c + 1) * USR].reshape(UT, P).T),
            "recipm": np.ascontiguousarray(
                recip_m[c * MSL:(c + 1) * MSL].reshape(MT, P).T),
            "ind": np.ascontiguousarray(ind_m[c * MSL:(c + 1) * MSL][None, :]),
        })
    return S, per_core, D_real


# ---------------- device program ----------------

def build_program(S):
    import os
    UPTO = int(os.environ.get('KUPTO', '9'))
    nc = bacc.Bacc("TRN2", target_bir_lowering=False, debug=False, num_devices=W)
    NB, NC, NDCH = S["NB"], S["NC"], S["NDCH"]
    B_chunks, C_chunks, D_chunks = S["B_chunks"], S["C_chunks"], S["D_chunks"]

    # ---- kernel I/O ----
    featsT = nc.dram_tensor("featsT", [FD, MSL], bf16, kind="ExternalInput")
    wm = nc.dram_tensor("wm", [FD, H], bf16, kind="ExternalInput")
    sq_names = ["wl1_mu", "wr1_um", "wl2_mu", "wl2_um", "wr2_mu", "wr2_um"]
    wt = {n: nc.dram_tensor(n, [H, H], bf16, kind="ExternalInput") for n in sq_names}
    row_names = ["fold1", "fold2", "foldm", "vrow", "bmh"]
    rw = {n: nc.dram_tensor(n, [1, H], bf16, kind="ExternalInput") for n in row_names}
    bm_col_in = nc.dram_tensor("bm_col", [P, 1], f32, kind="ExternalInput")
    ind_in = nc.dram_tensor("ind", [1, MSL], bf16, kind="ExternalInput")
    recipu_in = nc.dram_tensor("recipu", [P, UT], f32, kind="ExternalInput")
    recipm_in = nc.dram_tensor("recipm", [P, MT], f32, kind="ExternalInput")
    iota4_in = nc.dram_tensor("iota4", [P, 4 * P], bf16, kind="ExternalInput")
    iotac_in = nc.dram_tensor("iota_col", [P, 1], bf16, kind="ExternalInput")
    b_loc = nc.dram_tensor("b_loc", [P, NB], bf16, kind="ExternalInput")
    b_gidx = nc.dram_tensor("b_gidx", [P, NB * 8], i16, kind="ExternalInput")
    c_loc = nc.dram_tensor("c_loc", [P, NC], bf16, kind="ExternalInput")
    c_gidx = nc.dram_tensor("c_gidx", [P, NC * 8], i16, kind="ExternalInput")
    d_uidx = nc.dram_tensor("d_uidx", [P, NDCH * 8], i16, kind="ExternalInput")
    d_mloc = nc.dram_tensor("d_mloc", [1, NDCH * P], bf16, kind="ExternalInput")
    out = nc.dram_tensor("out", [P, NDCH], f32, kind="ExternalOutput")

    # ---- internal DRAM ----
    xcat_slice = nc.dram_tensor("xcat_slice", [MSL, 2 * H], bf16)
    xcat_full = nc.dram_tensor("xcat_full", [NMP, 2 * H], bf16, addr_space="Shared")
    userh = nc.dram_tensor("userh", [USR, H], bf16)
    usero = nc.dram_tensor("usero", [USR, H], bf16)
    partials = nc.dram_tensor("partials", [NMP, H], bf16)
    parts_recv = nc.dram_tensor("parts_recv", [NMP, H], bf16)
    mo_slice = nc.dram_tensor("mo_slice", [MSL, H], bf16)
    mo_full = nc.dram_tensor("mo_full", [NMP, H], bf16, addr_space="Shared")

    rg = [list(range(W))]
    RELU = mybir.ActivationFunctionType.Relu

    gsems = [nc.alloc_semaphore(f"gsem{i}") for i in range(16)]
    gcount = [0] * 16
    gptr = [0]

    def next_gsem(incs=1):
        i = gptr[0] % len(gsems)
        gptr[0] += 1
        gcount[i] += incs
        return gsems[i], gcount[i] * 16

    def dummy_gather(pool, gipool, table, elem, tag):
        gb0 = pool.tile([P, GROUP * elem], bf16, tag=tag + "b")
        gi0 = gipool.tile([P, GROUP * 8], i16, tag=tag + "i")
        nc.vector.memset(gi0[:], 0)
        gsem, gtgt = next_gsem()
        nc.gpsimd.dma_gather(
            out_ap=gb0[:].rearrange("p (c n) -> p c n", c=GROUP),
            in_ap=table, idxs_ap=gi0[:],
            num_idxs=GROUP * P, num_idxs_reg=GROUP * P,
            elem_size=elem, prepare_only=True, sem=gsem)
        nc.gpsimd.trigger_dma(count=None)
        nc.vector.tensor_copy(out=gb0[:, 0:2], in_=gb0[:, 0:2]).wait_op(gsem, gtgt, "sem-ge")

    from contextlib import ExitStack
    with tile.TileContext(nc) as tc, ExitStack() as stack:
        for s in gsems:
            nc.gpsimd.sem_clear(s)
        tc.strict_bb_all_engine_barrier()
        cst = stack.enter_context(tc.tile_pool(name="cst", bufs=1))

        # ---------- constants ----------
        iota4_t = cst.tile([P, 4 * P], bf16)
        nc.sync.dma_start(out=iota4_t[:], in_=iota4_in[:])
        iota_col = cst.tile([P, 1], bf16)
        nc.sync.dma_start(out=iota_col[:], in_=iotac_in[:])
        ones_row = cst.tile([1, P], bf16)
        nc.vector.memset(ones_row[:], 1.0)
        ones_col = cst.tile([P, 1], bf16)
        nc.vector.memset(ones_col[:], 1.0)
        ident_bf = cst.tile([P, P], bf16)
        make_identity(nc, ident_bf[:])
        wtile = {}
        for n in sq_names:
            t = cst.tile([P, P], bf16, tag=f"w_{n}")
            nc.sync.dma_start(out=t[:], in_=wt[n][:])
            wtile[n] = t
        rtile = {}
        for n in row_names:
            t = cst.tile([1, P], bf16, tag=f"r_{n}")
            nc.sync.dma_start(out=t[:], in_=rw[n][:])
            rtile[n] = t
        bm_col = cst.tile([P, 1], f32)
        nc.sync.dma_start(out=bm_col[:], in_=bm_col_in[:])
        recipu_t = cst.tile([P, UT], f32)
        nc.sync.dma_start(out=recipu_t[:], in_=recipu_in[:])
        recipm_t = cst.tile([P, MT], f32)
        nc.sync.dma_start(out=recipm_t[:], in_=recipm_in[:])
        mhT = cst.tile([P, MSL], bf16)        # movie_h feature-major, resident

        # ---------- Stage 0: movie tables ----------
        NCT = (MSL + 511) // 512
        with tc.tile_pool(name="s0_wm", bufs=1) as s0_wm, \
             tc.tile_pool(name="s0_ps", bufs=2, space="PSUM") as s0_ps, \
             tc.tile_pool(name="s0_p2", bufs=1, space="PSUM") as s0_p2, \
             tc.tile_pool(name="s0_ft", bufs=3) as s0_ft, \
             tc.tile_pool(name="s0_mx", bufs=2) as s0_mx, \
             tc.tile_pool(name="s0_sb", bufs=4) as s0_sb, \
             tc.tile_pool(name="s0_stg", bufs=3) as s0_stg:
            ind_t = s0_wm.tile([1, MSL], bf16)
            nc.sync.dma_start(out=ind_t[:], in_=ind_in[:])
            wm_t = []
            for k in range(4):
                t = s0_wm.tile([P, H], bf16, tag=f"wm{k}")
                nc.sync.dma_start(out=t[:], in_=wm[k * P:(k + 1) * P, :])
                wm_t.append(t)

            for j in range(NCT):
                c0 = j * 512
                cw = min(512, MSL - c0)
                nt = (cw + P - 1) // P
                mxps = s0_ps.tile([P, 512], f32, space="PSUM", tag="mx")
                for k in range(4):
                    ft = s0_ft.tile([P, 512], bf16, tag="ft")
                    nc.sync.dma_start(out=ft[:, :cw], in_=featsT[k * P:(k + 1) * P, c0:c0 + cw])
                    nc.tensor.matmul(out=mxps[:, :cw], lhsT=wm_t[k][:], rhs=ft[:, :cw],
                                     start=(k == 0), stop=(k == 3))
                # mxps is feature-major [h, m]; bm varies per feature h = partition,
                # so the +bm fold rides the PSUM->SBUF copy as a per-partition bias.
                mx_s = s0_mx.tile([P, 512], bf16, tag="mxs")
                nc.scalar.activation(out=mx_s[:, :cw], in_=mxps[:, :cw],
                                     func=mybir.ActivationFunctionType.Identity,
                                     bias=bm_col[:, 0:1])
                for tt in range(nt):
                    m0 = c0 + tt * P
                    mw = min(P, MSL - m0)
                    mx_sl = mx_s[:, tt * P: tt * P + mw]
                    ps1 = s0_p2.tile([P, P], f32, space="PSUM", tag="p1")
                    nc.tensor.matmul(out=ps1[:mw, :], lhsT=mx_sl, rhs=wtile["wl1_mu"][:],
                                     start=True, stop=False)
                    nc.tensor.matmul(out=ps1[:mw, :], lhsT=ones_row[:, :mw],
                                     rhs=rtile["fold1"][:], start=False, stop=True)
                    stg = s0_stg.tile([P, 2 * H], bf16, tag="xrow")
                    nc.scalar.copy(out=stg[:mw, 0:H], in_=ps1[:mw, :])
                    psA = s0_p2.tile([P, P], f32, space="PSUM", tag="pA")
                    nc.tensor.matmul(out=psA[:mw, :], lhsT=mx_sl, rhs=wtile["wr1_um"][:],
                                     start=True, stop=False)
                    nc.tensor.matmul(out=psA[:mw, :], lhsT=ones_row[:, :mw],
                                     rhs=rtile["bmh"][:], start=False, stop=False)
                    nc.tensor.matmul(out=psA[:mw, :], lhsT=ind_t[:, m0:m0 + mw],
                                     rhs=rtile["vrow"][:], start=False, stop=True)
                    mh_row = s0_sb.tile([P, P], bf16, tag="mhrow")
                    nc.scalar.activation(out=mh_row[:mw, :], in_=psA[:mw, :], func=RELU)
                    tp = s0_p2.tile([P, P], bf16, space="PSUM", tag="tp")
                    nc.tensor.transpose(out=tp[:], in_=mh_row[:], identity=ident_bf[:])
                    nc.scalar.copy(out=mhT[:, m0:m0 + mw], in_=tp[:, :mw])
                    ps2 = s0_p2.tile([P, P], f32, space="PSUM", tag="p2")
                    nc.tensor.matmul(out=ps2[:mw, :], lhsT=mhT[:, m0:m0 + mw],
                                     rhs=wtile["wl2_mu"][:], start=True, stop=False)
                    nc.tensor.matmul(out=ps2[:mw, :], lhsT=ones_row[:, :mw],
                                     rhs=rtile["fold2"][:], start=False, stop=True)
                    nc.scalar.copy(out=stg[:mw, H:2 * H], in_=ps2[:mw, :])
                    nc.sync.dma_start(out=xcat_slice[m0:m0 + mw, :], in_=stg[:mw, :])

        nc.gpsimd.collective_compute(
            "AllGather", mybir.AluOpType.bypass, replica_groups=rg,
            ins=[xcat_slice[:].opt()], outs=[xcat_full[:].opt()])
        tc.strict_bb_all_engine_barrier()

        # ---------- Pass B: user-side fused aggregation ----------
        if UPTO >= 2:
         with tc.tile_pool(name="pb_acc", bufs=1) as pb_acc, \
             tc.tile_pool(name="pb_sb", bufs=1) as pb_sb, \
             tc.tile_pool(name="pb_s4", bufs=4) as pb_s4, \
             tc.tile_pool(name="pb_g", bufs=3) as pb_g, \
             tc.tile_pool(name="pb_gi", bufs=3) as pb_gi, \
             tc.tile_pool(name="pb_ps", bufs=2, space="PSUM") as pb_ps, \
             tc.tile_pool(name="pb_pt", bufs=1, space="PSUM") as pb_pt, \
             tc.tile_pool(name="pb_stg", bufs=4) as pb_stg:
            bloc_t = pb_sb.tile([P, NB], bf16)
            nc.sync.dma_start(out=bloc_t[:], in_=b_loc[:])
            accB = pb_acc.tile([P, UT * 2 * H], bf16)
            # first range with chunks per tile (for copy-vs-add)
            first_r = {}
            for t in range(UT):
                rs = [r for r in range(NRNG) if B_chunks[r][t] > 0]
                first_r[t] = rs[0] if rs else None
                if not rs:
                    nc.vector.memset(accB[:, t * 2 * H:(t + 1) * 2 * H], 0.0)

            pos = 0
            for r in range(NRNG):
                table = xcat_full[RNG_STARTS[r]:RNG_ENDS[r], :]
                sub_chunks = int(B_chunks[r].sum())
                gpos = 0
                gbufs = []
                while gpos < sub_chunks:
                    gn = min(GROUP, sub_chunks - gpos)
                    gb = pb_g.tile([P, GROUP * 2 * H], bf16, tag="gbuf")
                    gi = pb_gi.tile([P, GROUP * 8], i16, tag="gidx")
                    col0 = (pos + gpos) * 8
                    nc.sync.dma_start(out=gi[:, :gn * 8], in_=b_gidx[:, col0: col0 + gn * 8])
                    nc.gpsimd.dma_gather(
                        out_ap=gb[:, :gn * 2 * H].rearrange("p (c n) -> p c n", c=gn),
                        in_ap=table,
                        idxs_ap=gi[:, :gn * 8],
                        num_idxs=gn * P,
                        num_idxs_reg=gn * P,
                        elem_size=2 * H,
                    )
                    gbufs.append((gpos, gn, gb, None, None))
                    gpos += gn

                def get_slot(sub_pos):
                    for g0, gn, gb, gsem, gtgt in gbufs:
                        if g0 <= sub_pos < g0 + gn:
                            return gb, sub_pos - g0, gsem, gtgt
                    raise AssertionError

                sub_pos = 0
                for t in range(UT):
                    n = int(B_chunks[r][t])
                    if n == 0:
                        continue
                    ps = pb_ps.tile([P, 2 * H], f32, space="PSUM", tag="ps")
                    for c4 in range((n + 3) // 4):
                        nch = min(4, n - c4 * 4)
                        s4 = pb_s4.tile([P, 4 * P], bf16, tag="s4")
                        cc = pos + sub_pos + c4 * 4
                        nc.vector.tensor_tensor(
                            out=s4[:].rearrange("p (k n) -> p k n", k=4),
                            in0=iota4_t[:].rearrange("p (k n) -> p k n", k=4),
                            in1=bloc_t[:, cc: cc + 4][:, :, None].to_broadcast([P, 4, P]),
                            op=mybir.AluOpType.is_equal,
                        )
                        for j in range(nch):
                            ch = c4 * 4 + j
                            gb, slot, gsem, gtgt = get_slot(sub_pos + ch)
                            nc.tensor.matmul(
                                out=ps[:],
                                lhsT=s4[:, j * P:(j + 1) * P],
                                rhs=gb[:, slot * 2 * H:(slot + 1) * 2 * H],
                                start=(ch == 0), stop=(ch == n - 1),
                            )
                    a0 = t * 2 * H
                    if first_r[t] == r:
                        nc.scalar.copy(out=accB[:, a0:a0 + 2 * H], in_=ps[:])
                    else:
                        nc.vector.tensor_tensor(out=accB[:, a0:a0 + 2 * H], in0=ps[:],
                                                in1=accB[:, a0:a0 + 2 * H],
                                                op=mybir.AluOpType.add)
                    sub_pos += n
                pos += sub_chunks

            # ---- epilogue: user_h / userh / user_o per tile ----
            for t in range(UT):
                a0 = t * 2 * H
                rc = recipu_t[:, t:t + 1]
                uh = pb_stg.tile([P, H], bf16, tag="uh")
                nc.scalar.activation(out=uh[:], in_=accB[:, a0:a0 + H], func=RELU,
                                     scale=rc)
                tp = pb_pt.tile([P, P], bf16, space="PSUM", tag="ep")
                nc.tensor.transpose(out=tp[:], in_=uh[:], identity=ident_bf[:])
                uhT = pb_stg.tile([P, P], bf16, tag="uhT")
                nc.scalar.copy(out=uhT[:], in_=tp[:])
                psU = pb_pt.tile([P, P], f32, space="PSUM", tag="ep2")
                nc.tensor.matmul(out=psU[:], lhsT=uhT[:], rhs=wtile["wl2_um"][:],
                                 start=True, stop=False)
                nc.tensor.matmul(out=psU[:], lhsT=ones_row[:],
                                 rhs=rtile["foldm"][:], start=False, stop=True)
                uh2 = pb_stg.tile([P, P], bf16, tag="uh2")
                nc.scalar.copy(out=uh2[:], in_=psU[:])
                nc.sync.dma_start(out=userh[t * P:(t + 1) * P, :], in_=uh2[:])
                psR = pb_pt.tile([P, P], f32, space="PSUM", tag="ep3")
                nc.tensor.matmul(out=psR[:], lhsT=uhT[:], rhs=wtile["wr2_mu"][:],
                                 start=True, stop=True)
                uo = pb_stg.tile([P, H], f32, tag="uo")
                nc.vector.tensor_scalar(out=uo[:], in0=accB[:, a0 + H:a0 + 2 * H],
                                        scalar1=rc, scalar2=None,
                                        op0=mybir.AluOpType.mult)
                uo_bf = pb_stg.tile([P, H], bf16, tag="uobf")
                nc.vector.tensor_tensor(out=uo_bf[:], in0=uo[:], in1=psR[:],
                                        op=mybir.AluOpType.add)
                nc.sync.dma_start(out=usero[t * P:(t + 1) * P, :], in_=uo_bf[:])

        # ---------- Pass C: movie-side aggregation of userh ----------
        if UPTO >= 3:
         tc.strict_bb_all_engine_barrier()
         with tc.tile_pool(name="pc_sb", bufs=1) as pc_sb, \
             tc.tile_pool(name="pc_s4", bufs=4) as pc_s4, \
             tc.tile_pool(name="pc_g", bufs=3) as pc_g, \
             tc.tile_pool(name="pc_gi", bufs=3) as pc_gi, \
             tc.tile_pool(name="pc_ps", bufs=2, space="PSUM") as pc_ps, \
             tc.tile_pool(name="pc_stg", bufs=4) as pc_stg:
            cloc_t = pc_sb.tile([P, NC], bf16)
            nc.sync.dma_start(out=cloc_t[:], in_=c_loc[:])

            gpos = 0
            gbufs = []
            while gpos < NC:
                gn = min(GROUP, NC - gpos)
                gb = pc_g.tile([P, GROUP * H], bf16, tag="gbuf")
                gi = pc_gi.tile([P, GROUP * 8], i16, tag="gidx")
                nc.sync.dma_start(out=gi[:, :gn * 8], in_=c_gidx[:, gpos * 8: (gpos + gn) * 8])
                nc.gpsimd.dma_gather(
                    out_ap=gb[:, :gn * H].rearrange("p (c n) -> p c n", c=gn),
                    in_ap=userh[:],
                    idxs_ap=gi[:, :gn * 8],
                    num_idxs=gn * P,
                    num_idxs_reg=gn * P,
                    elem_size=H,
                )
                gbufs.append((gpos, gn, gb, None, None))
                gpos += gn

            def get_slotC(p_):
                for g0, gn, gb, gsem, gtgt in gbufs:
                    if g0 <= p_ < g0 + gn:
                        return gb, p_ - g0, gsem, gtgt
                raise AssertionError

            pos = 0
            for g in range(GMT):
                n = int(C_chunks[g])
                stg = pc_stg.tile([P, H], bf16, tag="pstg")
                if n == 0:
                    nc.vector.memset(stg[:], 0.0)
                    nc.sync.dma_start(out=partials[g * P:(g + 1) * P, :], in_=stg[:])
                    continue
                ps = pc_ps.tile([P, H], f32, space="PSUM", tag="ps")
                for c4 in range((n + 3) // 4):
                    nch = min(4, n - c4 * 4)
                    s4 = pc_s4.tile([P, 4 * P], bf16, tag="s4")
                    cc = pos + c4 * 4
                    nc.vector.tensor_tensor(
                        out=s4[:].rearrange("p (k n) -> p k n", k=4),
                        in0=iota4_t[:].rearrange("p (k n) -> p k n", k=4),
                        in1=cloc_t[:, cc: cc + 4][:, :, None].to_broadcast([P, 4, P]),
                        op=mybir.AluOpType.is_equal,
                    )
                    for j in range(nch):
                        ch = c4 * 4 + j
                        gb, slot, gsem, gtgt = get_slotC(pos + ch)
                        nc.tensor.matmul(
                            out=ps[:],
                            lhsT=s4[:, j * P:(j + 1) * P],
                            rhs=gb[:, slot * H:(slot + 1) * H],
                            start=(ch == 0), stop=(ch == n - 1),
                        )
                nc.scalar.copy(out=stg[:], in_=ps[:])
                nc.sync.dma_start(out=partials[g * P:(g + 1) * P, :], in_=stg[:])
                pos += n

        if UPTO >= 3:
         nc.gpsimd.collective_compute(
            "AllToAll", mybir.AluOpType.bypass, replica_groups=rg,
            ins=[partials[:].opt()], outs=[parts_recv[:].opt()])
         tc.strict_bb_all_engine_barrier()

        # ---------- movie_o ----------
        if UPTO >= 4:
         with tc.tile_pool(name="mo_in", bufs=3) as mo_in, \
             tc.tile_pool(name="mo_ps", bufs=2, space="PSUM") as mo_ps, \
             tc.tile_pool(name="mo_stg", bufs=4) as mo_stg:
            rv = parts_recv[:].rearrange("(s m) h -> s m h", s=W)
            for t in range(MT):
                pin = mo_in.tile([P, W * H], bf16, tag="pin")
                nc.sync.dma_start(
                    out=pin[:].rearrange("p (s h) -> p s h", s=W),
                    in_=rv[:, t * P:(t + 1) * P, :].transpose([1, 0, 2]))
                s1 = mo_stg.tile([P, 4 * H], bf16, tag="s1")
                nc.vector.tensor_tensor(out=s1[:], in0=pin[:, 0:4 * H],
                                        in1=pin[:, 4 * H:8 * H], op=mybir.AluOpType.add)
                s2 = mo_stg.tile([P, 2 * H], bf16, tag="s2")
                nc.vector.tensor_tensor(out=s2[:], in0=s1[:, 0:2 * H],
                                        in1=s1[:, 2 * H:4 * H], op=mybir.AluOpType.add)
                s3 = mo_stg.tile([P, H], f32, tag="s3")
                nc.vector.tensor_tensor(out=s3[:], in0=s2[:, 0:H],
                                        in1=s2[:, H:2 * H], op=mybir.AluOpType.add)
                rps = mo_ps.tile([P, P], f32, space="PSUM", tag="mroot")
                nc.tensor.matmul(out=rps[:], lhsT=mhT[:, t * P:(t + 1) * P],
                                 rhs=wtile["wr2_um"][:], start=True, stop=True)
                mo_t = mo_stg.tile([P, H], f32, tag="mo1")
                nc.vector.tensor_scalar(out=mo_t[:], in0=s3[:],
                                        scalar1=recipm_t[:, t:t + 1], scalar2=None,
                                        op0=mybir.AluOpType.mult)
                mo_bf = mo_stg.tile([P, H], bf16, tag="mo2")
                nc.vector.tensor_tensor(out=mo_bf[:], in0=mo_t[:], in1=rps[:],
                                        op=mybir.AluOpType.add)
                nc.sync.dma_start(out=mo_slice[t * P:(t + 1) * P, :], in_=mo_bf[:])

        if UPTO >= 4:
         nc.gpsimd.collective_compute(
            "AllGather", mybir.AluOpType.bypass, replica_groups=rg,
            ins=[mo_slice[:].opt()], outs=[mo_full[:].opt()])
         tc.strict_bb_all_engine_barrier()

        # ---------- Pass D: label dots ----------
        if UPTO < 5:
            with tc.tile_pool(name="dummy", bufs=1) as dp:
                z = dp.tile([P, NDCH], f32)
                nc.vector.memset(z[:], 0.0)
                nc.sync.dma_start(out=out[:], in_=z[:])
        # chunk -> movie tile mapping
        chunk_tile = []
        for g in range(GMT):
            chunk_tile += [g] * int(D_chunks[g])
        assert len(chunk_tile) == NDCH

        if UPTO >= 5:
         with tc.tile_pool(name="pd_g", bufs=3) as pd_g, \
             tc.tile_pool(name="pd_gi", bufs=3) as pd_gi, \
             tc.tile_pool(name="pd_mo", bufs=3) as pd_mo, \
             tc.tile_pool(name="pd_loc", bufs=3) as pd_loc, \
             tc.tile_pool(name="pd_out", bufs=1) as pd_out, \
             tc.tile_pool(name="pd_pl", bufs=2, space="PSUM") as pd_pl, \
             tc.tile_pool(name="pd_pm", bufs=2, space="PSUM") as pd_pm, \
             tc.tile_pool(name="pd_sb", bufs=4) as pd_sb:
            outstrip = pd_out.tile([P, NDCH], f32)
            last_tile = [None]
            mo_hold = [None]
            gpos = 0
            while gpos < NDCH:
                gn = min(GROUP, NDCH - gpos)
                ut = pd_g.tile([P, GROUP * H], bf16, tag="ut")
                gi = pd_gi.tile([P, GROUP * 8], i16, tag="gidx")
                nc.sync.dma_start(out=gi[:, :gn * 8], in_=d_uidx[:, gpos * 8:(gpos + gn) * 8])
                nc.gpsimd.dma_gather(
                    out_ap=ut[:, :gn * H].rearrange("p (c n) -> p c n", c=gn),
                    in_ap=usero[:],
                    idxs_ap=gi[:, :gn * 8],
                    num_idxs=gn * P,
                    num_idxs_reg=gn * P,
                    elem_size=H,
                )
                loc_sb = pd_loc.tile([1, GROUP * P], bf16, tag="loc")
                nc.sync.dma_start(out=loc_sb[:, :gn * P],
                                  in_=d_mloc[:, gpos * P:(gpos + gn) * P])

                for q0 in range(0, gn, 4):
                    qn = min(4, gn - q0)
                    ps_loc = pd_pl.tile([P, 4 * P], f32, space="PSUM", tag="psl")
                    nc.tensor.matmul(out=ps_loc[:, :qn * P], lhsT=ones_row[:],
                                     rhs=loc_sb[:, q0 * P:(q0 + qn) * P],
                                     start=True, stop=True)
                    locb = pd_sb.tile([P, 4 * P], bf16, tag="locb")
                    nc.scalar.copy(out=locb[:, :qn * P], in_=ps_loc[:, :qn * P])
                    S_t = pd_sb.tile([P, 4 * P], bf16, tag="S")
                    nc.vector.tensor_tensor(
                        out=S_t[:, :qn * P], in0=locb[:, :qn * P],
                        in1=iota_col[:, 0:1].to_broadcast([P, qn * P]),
                        op=mybir.AluOpType.is_equal)
                    for q in range(qn):
                        ch = gpos + q0 + q
                        g = chunk_tile[ch]
                        if g != last_tile[0]:
                            mo_t = pd_mo.tile([P, P], bf16, tag="mot")
                            mo_hold[0] = mo_t
                            nc.sync.dma_start(out=mo_t[:],
                                              in_=mo_full[g * P:(g + 1) * P, :])
                            last_tile[0] = g
                        psM = pd_pm.tile([P, P], f32, space="PSUM", tag="psM")
                        nc.tensor.matmul(out=psM[:], lhsT=S_t[:, q * P:(q + 1) * P],
                                         rhs=mo_hold[0][:], start=True, stop=True)
                        prod = pd_sb.tile([P, H], f32, tag="prod")
                        nc.vector.tensor_tensor(
                            out=prod[:], in0=ut[:, (q0 + q) * H:(q0 + q + 1) * H],
                            in1=psM[:], op=mybir.AluOpType.mult,
                        )
                        nc.vector.tensor_reduce(
                            out=outstrip[:, ch:ch + 1], in_=prod[:],
                            axis=mybir.AxisListType.X, op=mybir.AluOpType.add)
                gpos += gn
            nc.sync.dma_start(out=out[:], in_=outstrip[:])

    nc.compile()
    return nc


# ---------------- entry point ----------------

_CACHE = {}
TRACE = False
LAST_EXEC_NS = None
LAST_RESULTS = None


def kernel(movie_feats, user_init, edge_src, edge_dst, lbl_user, lbl_movie, n_users,
           Wm, bm,
           Wl1_um, bl1_um, Wr1_um, Wl1_mu, bl1_mu, Wr1_mu,
           Wl2_um, bl2_um, Wr2_um, Wl2_mu, bl2_mu, Wr2_mu):
    movie_feats = np.asarray(movie_feats, dtype=np.float32)
    u0 = np.asarray(user_init, np.float32)
    S, per_core, D_real = preprocess(edge_src, edge_dst, lbl_user, lbl_movie)

    key = (S["NB"], S["NC"], S["NDCH"],
           S["B_chunks"].tobytes(), S["C_chunks"].tobytes(), S["D_chunks"].tobytes())
    if key in _CACHE:
        nc = _CACHE[key]
    else:
        nc = build_program(S)
        _CACHE[key] = nc

    featsT = np.zeros((FD, NMP), npbf16)
    featsT[:, :NM] = movie_feats.T.astype(npbf16)

    def rowb(x):
        return np.ascontiguousarray(np.asarray(x, np.float32)[None, :].astype(npbf16))

    fold1 = np.asarray(bl1_mu, np.float32) + u0 @ np.asarray(Wr1_mu, np.float32)
    vrow = u0 @ np.asarray(Wl1_um, np.float32)

    weights = {
        "wm": np.asarray(Wm, np.float32).astype(npbf16),
        "wl1_mu": np.asarray(Wl1_mu, np.float32).astype(npbf16),
        "wr1_um": np.asarray(Wr1_um, np.float32).astype(npbf16),
        "wl2_mu": np.asarray(Wl2_mu, np.float32).astype(npbf16),
        "wl2_um": np.asarray(Wl2_um, np.float32).astype(npbf16),
        "wr2_mu": np.asarray(Wr2_mu, np.float32).astype(npbf16),
        "wr2_um": np.asarray(Wr2_um, np.float32).astype(npbf16),
        "bm_col": np.ascontiguousarray(np.asarray(bm, np.float32)[:, None]),
        "fold1": rowb(fold1),
        "fold2": rowb(bl2_mu),
        "foldm": rowb(bl2_um),
        "vrow": rowb(vrow),
        "bmh": rowb(bl1_um),
    }

    in_maps = []
    for c in range(W):
        m = {"featsT": np.ascontiguousarray(featsT[:, c * MSL:(c + 1) * MSL])}
        m.update(weights)
        m.update(per_core[c])
        in_maps.append(m)

    global LAST_EXEC_NS, LAST_RESULTS
    res = run_bass_kernel_spmd(nc, in_maps, core_ids=list(range(W)), trace=TRACE)
    LAST_EXEC_NS = res.exec_time_ns
    LAST_RESULTS = res

    EL = len(np.asarray(lbl_user))
    out_full = np.zeros(EL, np.float32)
    for c in range(W):
        vals = res.results[c]["out"].T.reshape(-1)     # stream order
        real = D_real[c]
        mask = real >= 0
        out_full[real[mask]] = vals[mask]
    return out_full
